# revision 32
# baseline (speedup 1.0000x reference)
"""Trainium2 Bass kernel for nn_AdaptivePrototypeRefiner.

8-core SPMD, data-parallel over the N_q axis (2048 queries/core); weights and
prototypes replicated.  All compute in bf16 on the TensorEngine with fp32 PSUM
accumulation (rel err ~1.5e-4 vs the fp32 reference).

Math/structure (per reference):
  * refinement loop (3 steps): soft = softmax(-d/T) depends only on the input
    distances, NOT on the running `refined` — so all 3 steps' weighted sums
    (soft.T @ qf and soft.sum(0)) are computed locally in one pass, AllReduced
    once (bf16 wire, 98KB), and the tiny 2-layer MLP chain then runs
    replicated on every core while the confidence phase owns the engines.
  * confidence: conf[k] = mean_n sigmoid(relu(hp[k] + hq[n] + bc1) @ Wc2 + bc2).
    hq = qf @ Wq on PE; the (K, N_q/8, 256) relu tensor is produced per
    (class, h-half) as (128h, 2048n) bf16 tiles by DVE dual-op tensor_scalar
    (relu(x+bias), 4x mode) and ACT activation (3:1 split); the Wc2-weighted
    h-reduction runs on PE as accumulating matmuls with a diag-expanded Wc2
    stationary, col-tiled 4 ways (tile_position) so four classes' matmuls
    overlap in the array.  sigmoid-mean uses sigmoid(z) = 0.5 + 0.5*tanh(z/2):
    one ACT Tanh op per psum chunk with accum_out produces the partial sums,
    AllReduced (512B) after AllReduce #1 completes, finished with an affine
    on gpsimd.
  Host side does layout staging only: sharding, bf16 casts, transposes,
  diag-expansion of Wc2, and the inverse class permutation of the conf output.

Measured on the axon-tunneled chip: ~128-139 us NEFF exec (best 120.5), of
which ~30 us is core-start skew absorbed by AllReduce #1, ~25 us per-collective
ncfw overhead x2, and ~10 us kernel drain: local compute finishes by ~72 us;
the tail is the replicated MLP chain (on a HAM-throttled cold PE) overlapped
with AllReduce #2.  Step 0's prototype-half of the first MLP layer is
pre-accumulated during the AllReduce wait; per-step wmean transposes are
emitted inside the step loop so they fill step-boundary PE gaps.
"""

import sys

for _p in ("/opt/trn_rl_repo",):
    if _p not in sys.path:
        sys.path.append(_p)

import numpy as np
import ml_dtypes

import concourse.bass as bass
import concourse.bacc as bacc
import concourse.mybir as mybir
import concourse.tile as tile
from concourse.tile_rust import add_dep_helper
from concourse.bass_utils import run_bass_kernel_spmd

F32 = mybir.dt.float32
BF16 = mybir.dt.bfloat16
BF = ml_dtypes.bfloat16

N_CORES = 8
K = 32          # classes
NQ = 16384      # queries total
C = 512         # feature dim
HR = 512        # refinement hidden
HC = 256        # confidence hidden
S = 3           # refinement steps
NQS = NQ // N_CORES   # 2048 queries per core
NB = NQS // 128       # 16 blocks of 128 queries
NCH = NQS // 512      # 4 chunks of 512 queries (psum-bank sized)

TRACE = False           # set by test harness for profiling runs
TRACE_KWARGS = {}

_CACHE = {}

AF = mybir.ActivationFunctionType
OP = mybir.AluOpType


def _build(gate_t2=True):
    nc = bacc.Bacc("TRN2", target_bir_lowering=False, debug=False,
                   num_devices=N_CORES)

    # ---------------- dram parameters (per-core shards / replicated) ---------
    P = lambda name, shape, dt: nc.declare_dram_parameter(name, list(shape), dt, isOutput=False)
    qfn_d = P("qfn", (128, NB, C), BF16)          # qf shard, n = p*16+g
    qft_d = P("qft", (128, C // 128, NQS), BF16)  # qf shard transposed [c%128, c//128, (g,p)]
    dist_d = P("dist", (128, NB, K), F32)         # distances shard
    w1_d = P("w1", (128, 8, HR), BF16)            # W1 [cc%128, cc//128, h]
    w2_d = P("w2", (128, 4, C), BF16)             # W2 [h%128, h//128, c]
    wq_d = P("wq", (128, 4, HC), BF16)            # Wc1[512:] blocks
    wp_d = P("wp", (128, 4, HC), BF16)            # Wc1[:512] blocks
    w2d_d = P("w2d", (128, 2, K, K), BF16)        # diag-expanded Wc2
    ptT_d = P("ptT", (128, 4, K), BF16)           # prototypes^T blocks
    pt_d = P("pt", (K, C), F32)                   # prototypes (refined init)
    b1_d = P("b1r", (1, HR), BF16)
    b2_d = P("b2r", (1, C), BF16)
    bc1_d = P("bc1r", (1, HC), BF16)
    bc2_d = P("bc2c", (128, 1), F32)              # bc2 replicated down partitions
    id_d = P("ident", (128, 128), BF16)
    idf_d = P("identf", (128, 128), F32)

    ref_out = nc.declare_dram_parameter("refined", [K, C], F32, isOutput=True)
    conf_out = nc.declare_dram_parameter("conf", [128, 1], F32, isOutput=True)

    with tile.TileContext(nc) as tc:
        with (
            tc.tile_pool(name="sbuf", bufs=1) as sb,
            tc.tile_pool(name="rpool", bufs=10) as rp,
            tc.tile_pool(name="psA", bufs=4, space="PSUM") as psA,
            tc.tile_pool(name="psH", bufs=2, space="PSUM") as psH,
            tc.tile_pool(name="psG", bufs=2, space="PSUM") as psG,
            tc.tile_pool(name="dram", bufs=1, space="DRAM") as dram,
        ):
            # PSUM budget (8 banks): conf_ps 4 + hq_ps 2 + ps_gen 2
            gen_tile = lambda shape, dt: psG.tile(list(shape), dt, tag="ps_gen",
                                                  name="ps_gen")
            # ------------- input DMA to SBUF ---------------------------------
            def load(dparam, shape, dt, name):
                t = sb.tile(list(shape), dt, tag=name)
                nc.sync.dma_start(out=t[:], in_=dparam[:])
                return t

            dist = load(dist_d, (128, NB, K), F32, "dist")
            qfn = sb.tile([128, NB, C], BF16, tag="qfn", name="qfn")
            for _c in range(4):
                nc.sync.dma_start(out=qfn[:, 4 * _c:4 * (_c + 1), :],
                                  in_=qfn_d[:, 4 * _c:4 * (_c + 1), :])
            qft = sb.tile([128, 4, NQS], BF16, tag="qft", name="qft")
            for _c in range(4):
                nc.sync.dma_start(out=qft[:, :, 512 * _c:512 * (_c + 1)],
                                  in_=qft_d[:, :, 512 * _c:512 * (_c + 1)])
            wq = load(wq_d, (128, 4, HC), BF16, "wq")
            wp = load(wp_d, (128, 4, HC), BF16, "wp")
            ptT = load(ptT_d, (128, 4, K), BF16, "ptT")
            bc1r = load(bc1_d, (1, HC), BF16, "bc1r")
            bc2c = load(bc2_d, (128, 1), F32, "bc2c")
            w2d = load(w2d_d, (128, 2, K, K), BF16, "w2d")
            ident = load(id_d, (128, 128), BF16, "ident")
            identf = load(idf_d, (128, 128), F32, "identf")
            w1 = load(w1_d, (128, 8, HR), BF16, "w1")
            w2 = load(w2_d, (128, 4, C), BF16, "w2")
            b1r = load(b1_d, (1, HR), BF16, "b1r")
            b2r = load(b2_d, (1, C), BF16, "b2r")

            ones_row = sb.tile([1, K], BF16, tag="ones_row")
            nc.vector.memset(ones_row[:], 1.0)
            ones_col = sb.tile([128, 1], BF16, tag="ones_col")
            nc.vector.memset(ones_col[:], 1.0)

            # ------------- stage A: softmax + weighted-sum partials ----------
            esb = sb.tile([128, NB, S, K], F32, tag="esb")
            for s in range(S):
                nc.scalar.activation(esb[:, :, s, :], dist[:], AF.Exp,
                                     scale=-1.0 / (s + 1.0))
            zsb = sb.tile([128, NB, S], F32, tag="zsb")
            rz = sb.tile([128, NB, S], F32, tag="rz")
            soft = sb.tile([128, NB, S, K], BF16, tag="soft")
            for s in range(S):
                nc.vector.tensor_reduce(zsb[:, :, s], esb[:, :, s, :],
                                        axis=mybir.AxisListType.X, op=OP.add)
                nc.vector.reciprocal(rz[:, :, s], zsb[:, :, s])
                rz_ap = rz[:, :, s:s + 1]
                rz_b = bass.AP(rz_ap.tensor, rz_ap.offset,
                               [rz_ap.ap[0], rz_ap.ap[1], [0, K]])
                nc.vector.tensor_tensor(out=soft[:, :, s, :],
                                        in0=esb[:, :, s, :], in1=rz_b,
                                        op=OP.mult)

            num_ps = gen_tile([S * K, C], F32)
            wsum_ps = gen_tile([S * K, 1], F32)
            for g in range(NB):
                nc.tensor.matmul(wsum_ps[:], soft[:, g], ones_col[:],
                                 start=(g == 0), stop=(g == NB - 1))
            for g in range(NB):
                nc.tensor.matmul(num_ps[:], soft[:, g], qfn[:, g, :],
                                 start=(g == 0), stop=(g == NB - 1))

            # pack [num | wsum] into one bf16 staging tile -> one DMA, and
            # AllReduce on half the bytes (bf16 CCE adds are plenty accurate
            # for a 2e-2 tolerance).
            nw_sb = sb.tile([S * K, C + 1], BF16, tag="nw_sb")
            nc.vector.tensor_copy(nw_sb[:, C:], wsum_ps[:])
            nc.vector.tensor_copy(nw_sb[:, :C], num_ps[:])

            # ------------- AllReduce #1 (98KB) -------------------------------
            ar1_in = dram.tile([S * K, C + 1], BF16)
            ar1_out = dram.tile([S * K, C + 1], BF16)
            nc.gpsimd.dma_start(out=ar1_in[:], in_=nw_sb[:])
            nc.gpsimd.collective_compute(
                "AllReduce", OP.add,
                replica_groups=[list(range(N_CORES))],
                ins=[ar1_in.opt()], outs=[ar1_out.opt()],
            )
            arn = sb.tile([S * K, C + 1], BF16, tag="arn")
            _arn_i = nc.sync.dma_start(out=arn[:], in_=ar1_out[:])

            # ------------- confidence setup: hp, hq --------------------------
            # hpT blocks (128h, K) = Wp_blk.T @ protoT_blk + bc1
            hpbT = sb.tile([128, 2, K], F32, tag="hpbT")
            for hb in range(2):
                hp_ps = gen_tile([128, K], F32)
                for cb in range(4):
                    nc.tensor.matmul(hp_ps[:],
                                     wp[:, cb, hb * 128:(hb + 1) * 128],
                                     ptT[:, cb, :],
                                     start=(cb == 0), stop=False)
                nc.tensor.matmul(hp_ps[:], bc1r[:, hb * 128:(hb + 1) * 128],
                                 ones_row[:], start=False, stop=True)
                nc.vector.tensor_copy(hpbT[:, hb, :], hp_ps[:])

            # hqT (h-part, n-free) bf16
            hqt = sb.tile([128, 2, NQS], BF16, tag="hqt")
            for hb in range(2):
                for j in range(NCH):
                    hq_ps = psH.tile([128, 512], F32, tag="hq_ps")
                    for cb in range(4):
                        nc.tensor.matmul(hq_ps[:],
                                         wq[:, cb, hb * 128:(hb + 1) * 128],
                                         qft[:, cb, j * 512:(j + 1) * 512],
                                         start=(cb == 0), stop=(cb == 3))
                    nc.vector.tensor_copy(hqt[:, hb, j * 512:(j + 1) * 512],
                                          hq_ps[:])

            # ------------- confidence main loop ------------------------------
            conf_ps = [psA.tile([128, 512], F32, tag="conf_ps", name=f"conf_ps{_j}")
                       for _j in range(NCH)]
            # col-tiled: class k -> PE col-group g=k%4, psum rows 32g..32g+31
            # (its row within the group is k//4, set by the diag layout of w2d).
            # Emission interleaves the 4 col-groups so their matmuls overlap
            # in the array.
            last_dve_relu = last_act_relu = last_conf_mm = None
            cnt = {}
            for hb in range(2):
                for kq in range(0, K, 4):
                    qi = hb * 8 + kq // 4    # quad index 0..15
                    rts = []
                    for dk in range(4):
                        k = kq + dk
                        rt = rp.tile([128, NQS], BF16, tag="rt", name="rt")
                        if dk != qi % 4:
                            last_dve_relu = nc.vector.tensor_scalar(
                                out=rt[:], in0=hqt[:, hb, :],
                                scalar1=hpbT[:, hb, k:k + 1], scalar2=0.0,
                                op0=OP.add, op1=OP.max)
                        else:
                            last_act_relu = nc.scalar.activation(
                                rt[:], hqt[:, hb, :], AF.Relu,
                                bias=hpbT[:, hb, k:k + 1])
                        rts.append(rt)
                    for j in range(NCH):
                        for dk in range(4):
                            k = kq + dk
                            g = k % 4
                            c = cnt.get((j, g), 0)
                            last_conf_mm = nc.tensor.matmul(
                                conf_ps[j][32 * g:32 * (g + 1), :],
                                w2d[:, hb, k, :],
                                rts[dk][:, j * 512:(j + 1) * 512],
                                start=(c == 0),
                                stop=(c == 2 * (K // 4) - 1),
                                tile_position=(0, 32 * g),
                                skip_group_check=True)
                            cnt[(j, g)] = c + 1

            # tanh( 0.5*raw + 0.5*bc2 ), accumulate over n
            half_bc2 = sb.tile([128, 1], F32, tag="half_bc2")
            nc.vector.tensor_scalar(out=half_bc2[:], in0=bc2c[:], scalar1=0.5,
                                    scalar2=None, op0=OP.mult)
            th_scr = sb.tile([128, 512], BF16, tag="th_scr")
            tsum = sb.tile([128, NCH], F32, tag="tsum")
            for j in range(NCH):
                nc.scalar.activation(th_scr[:], conf_ps[j][:], AF.Tanh,
                                     bias=half_bc2[:], scale=0.5,
                                     accum_out=tsum[:, j:j + 1])
            tsm = sb.tile([128, 1], F32, tag="tsm")
            _tsm_i = nc.vector.tensor_reduce(tsm[:], tsum[:],
                                             axis=mybir.AxisListType.X,
                                             op=OP.add)

            # ------------- AllReduce #2: issue from DVE so it is not blocked
            # behind AR1 on the gpsimd FIFO -------------------------------
            ar2_in = dram.tile([128, 1], F32)
            ar2_out = dram.tile([128, 1], F32)
            nc.gpsimd.dma_start(out=ar2_in[:], in_=tsm[:])
            _t2_i = nc.gpsimd.collective_compute(
                "AllReduce", OP.add,
                replica_groups=[list(range(N_CORES))],
                ins=[ar2_in.opt()], outs=[ar2_out.opt()],
            )
            # fire the 2nd collective only once the 1st has fully completed
            # (an early doorbell mid-collective was once observed to stall).
            if gate_t2:
                add_dep_helper(_t2_i.ins, _arn_i.ins, sync=True,
                               reason="trigger AR2 after AR1 completion")
            ar2_sb = sb.tile([128, 1], F32, tag="ar2_sb")
            nc.sync.dma_start(out=ar2_sb[:], in_=ar2_out[:])
            conf_f = sb.tile([128, 1], F32, tag="conf_f")
            nc.gpsimd.tensor_scalar(out=conf_f[:], in0=ar2_sb[:],
                                    scalar1=0.5 / NQ, scalar2=0.5,
                                    op0=OP.mult, op1=OP.add)
            _confdma_i = nc.sync.dma_start(out=conf_out[:], in_=conf_f[:])

            # ------------- refinement MLP chain (after AR1) ------------------
            wsc = sb.tile([S * K, 1], F32, tag="wsc")
            _wsc_i = nc.vector.tensor_scalar(out=wsc[:], in0=arn[:, C:],
                                             scalar1=1e-6,
                                             scalar2=None, op0=OP.max)
            add_dep_helper(_wsc_i.ins, last_dve_relu.ins, sync=False,
                           reason="keep AR1-gated DVE chain after relu stream")
            add_dep_helper(_wsc_i.ins, _tsm_i.ins, sync=False,
                           reason="AR2-feeding reduce before AR1-gated DVE chain")
            rws = sb.tile([S * K, 1], F32, tag="rws")
            nc.vector.reciprocal(rws[:], wsc[:])
            wmean = sb.tile([S * K, C], BF16, tag="wmean")
            nc.vector.tensor_scalar(out=wmean[:], in0=arn[:, :C],
                                    scalar1=rws[:], scalar2=None, op0=OP.mult)
            wmT = sb.tile([128, S, 4, K], BF16, tag="wmT", name="wmT")

            def emit_wmT(s):
                for cb in range(4):
                    tr_ps = gen_tile([128, K], BF16)
                    _t_i = nc.tensor.transpose(
                        tr_ps[:],
                        wmean[s * K:(s + 1) * K, cb * 128:(cb + 1) * 128],
                        ident[s * K:(s + 1) * K, s * K:(s + 1) * K])
                    if s == 0 and cb == 0:
                        add_dep_helper(_t_i.ins, last_conf_mm.ins, sync=False,
                                       reason="MLP PE work after conf stream")
                    nc.vector.tensor_copy(wmT[:, s, cb, :], tr_ps[:])

            # Step-0 refined half of the first MLP layer: refined_0 ==
            # prototypes, so these matmuls have no AR1 dependency — run them
            # during the AR1 wait; the wmean half accumulates on top later.
            h_ps0 = psH.tile([K, HR], F32, tag="hq_ps", name="h_ps0")
            for cb in range(4):
                nc.tensor.matmul(h_ps0[:], ptT[:, cb, :], w1[:, cb, :],
                                 start=(cb == 0), stop=False)
            nc.tensor.matmul(h_ps0[:], ones_row[:], b1r[:],
                             start=False, stop=False)

            ref_f = []
            for s in range(S + 1):
                ref_f.append(sb.tile([K, C], F32, tag=f"ref_f{s}", name=f"ref_f{s}"))
            nc.sync.dma_start(out=ref_f[0][:], in_=pt_d[:])

            for s in range(S):
                emit_wmT(s)
                if s == 0:
                    # refined half + b1 already accumulated in h_ps0
                    h_ps = h_ps0
                else:
                    # concat^T refined blocks: transpose the f32 state
                    # directly (fp32 PE transpose), cast in the psum->sbuf
                    # copy — no separate bf16 cast of refined needed.
                    catT = sb.tile([128, 4, K], BF16, tag="catT", name="catT")
                    for cb in range(4):
                        tr_ps = gen_tile([128, K], F32)
                        nc.tensor.transpose(
                            tr_ps[:],
                            ref_f[s][:, cb * 128:(cb + 1) * 128],
                            identf[:K, :K])
                        nc.vector.tensor_copy(catT[:, cb, :], tr_ps[:])
                    h_ps = gen_tile([K, HR], F32)
                    for cb in range(4):
                        nc.tensor.matmul(h_ps[:], catT[:, cb, :],
                                         w1[:, cb, :],
                                         start=(cb == 0), stop=False)
                    nc.tensor.matmul(h_ps[:], ones_row[:], b1r[:],
                                     start=False, stop=False)
                for cb in range(4):
                    nc.tensor.matmul(h_ps[:], wmT[:, s, cb, :],
                                     w1[:, 4 + cb, :],
                                     start=False, stop=(cb == 3))
                h_bf = sb.tile([K, HR], BF16, tag="h_bf")
                _hr_i = nc.scalar.activation(h_bf[:], h_ps[:], AF.Relu)
                if s == 0:
                    add_dep_helper(_hr_i.ins, last_act_relu.ins, sync=False,
                                   reason="MLP ACT work after relu stream")
                # hT blocks
                hT = sb.tile([128, 4, K], BF16, tag="hT")
                for cb in range(4):
                    tr_ps = gen_tile([128, K], BF16)
                    nc.tensor.transpose(tr_ps[:],
                                        h_bf[:, cb * 128:(cb + 1) * 128],
                                        ident[:K, :K])
                    nc.vector.tensor_copy(hT[:, cb, :], tr_ps[:])
                # refinement = h @ W2 + b2 ; refined += 0.1*refinement
                rf_ps = gen_tile([K, C], F32)
                for cb in range(4):
                    nc.tensor.matmul(rf_ps[:], hT[:, cb, :], w2[:, cb, :],
                                     start=(cb == 0), stop=False)
                nc.tensor.matmul(rf_ps[:], ones_row[:], b2r[:],
                                 start=False, stop=True)
                nc.vector.scalar_tensor_tensor(out=ref_f[s + 1][:],
                                               in0=rf_ps[:], scalar=0.1,
                                               in1=ref_f[s][:],
                                               op0=OP.mult, op1=OP.add)
            _refdma_i = nc.sync.dma_start(out=ref_out[:], in_=ref_f[S][:])
            add_dep_helper(_refdma_i.ins, _confdma_i.ins, sync=False,
                           reason="conf output DMA ahead of refined in sync FIFO")

    nc.compile()
    return nc


def _prep_inputs(prototypes, query_features, query_distances,
                 W1, b1, W2, b2, Wc1, bc1, Wc2, bc2):
    """Host-side sharding + layout staging (no cross-tensor arithmetic)."""
    f32 = np.float32
    qf = np.asarray(query_features, f32)
    qd = np.asarray(query_distances, f32)
    W1 = np.asarray(W1, f32); b1 = np.asarray(b1, f32)
    W2 = np.asarray(W2, f32); b2 = np.asarray(b2, f32)
    Wc1 = np.asarray(Wc1, f32); bc1 = np.asarray(bc1, f32)
    Wc2 = np.asarray(Wc2, f32); bc2 = np.asarray(bc2, f32)
    pt = np.asarray(prototypes, f32)

    def blk(a, nb):  # (nb*128, m) -> (128, nb, m)
        n, m = a.shape
        return np.ascontiguousarray(
            a.reshape(nb, 128, m).transpose(1, 0, 2)).astype(BF)

    shared = {
        "w1": blk(W1, 8),
        "w2": blk(W2, 4),
        "wq": blk(Wc1[C:], 4),
        "wp": blk(Wc1[:C], 4),
        "ptT": blk(np.ascontiguousarray(pt.T), 4),
        "pt": pt,
        "b1r": b1.reshape(1, HR).astype(BF),
        "b2r": b2.reshape(1, C).astype(BF),
        "bc1r": bc1.reshape(1, HC).astype(BF),
        "bc2c": np.full((128, 1), float(bc2.reshape(-1)[0]), f32),
        "ident": np.eye(128, dtype=f32).astype(BF),
        "identf": np.eye(128, dtype=f32),
    }
    w2dm = np.zeros((128, 2, K, K), f32)
    for hb in range(2):
        for kk in range(K):
            w2dm[:, hb, kk, kk // 4] = Wc2[hb * 128:(hb + 1) * 128, 0]
    shared["w2d"] = w2dm.astype(BF)

    in_maps = []
    for s in range(N_CORES):
        qf_sh = qf[s * NQS:(s + 1) * NQS]          # (2048, 512)
        qd_sh = qd[s * NQS:(s + 1) * NQS]          # (2048, 32)
        qfn = qf_sh.reshape(128, NB, C).astype(BF)  # n = p*16+g
        # qft[c%128, c//128, g*128+p] = qf_sh[p*16+g, c]
        qft = np.ascontiguousarray(
            qf_sh.reshape(128, NB, C).transpose(2, 1, 0)  # (C, NB, 128)
            .reshape(C, NQS).reshape(4, 128, NQS).transpose(1, 0, 2)
        ).astype(BF)
        dist = np.ascontiguousarray(qd_sh.reshape(128, NB, K))
        m = dict(shared)
        m.update({"qfn": qfn, "qft": qft, "dist": dist})
        in_maps.append(m)
    return in_maps


def kernel(**inputs):
    if "nc" not in _CACHE:
        _CACHE["nc"] = _build()
    nc = _CACHE["nc"]
    in_maps = _prep_inputs(**inputs)
    res = run_bass_kernel_spmd(nc, in_maps, core_ids=list(range(N_CORES)),
                               trace=TRACE, **TRACE_KWARGS)
    _CACHE["last_result"] = res
    refined = np.asarray(res.results[0]["refined"], np.float32).reshape(K, C)
    conf_raw = np.asarray(res.results[0]["conf"], np.float32).reshape(128)
    kk = np.arange(K)
    conf = conf_raw[32 * (kk % 4) + kk // 4]
    return refined, conf


# revision 36
# speedup vs baseline: 1.0698x; 1.0698x over previous
"""Trainium2 Bass kernel for nn_AdaptivePrototypeRefiner.

8-core SPMD, data-parallel over the N_q axis (2048 queries/core); weights and
prototypes replicated.  All compute in bf16 on the TensorEngine with fp32 PSUM
accumulation (rel err ~1.5e-4 vs the fp32 reference).

Math/structure (per reference):
  * refinement loop (3 steps): soft = softmax(-d/T) depends only on the input
    distances, NOT on the running `refined` — so all 3 steps' weighted sums
    (soft.T @ qf and soft.sum(0)) are computed locally in one pass, AllReduced
    once (bf16 wire, 98KB), and the tiny 2-layer MLP chain then runs
    replicated on every core while the confidence phase owns the engines.
  * confidence: conf[k] = mean_n sigmoid(relu(hp[k] + hq[n] + bc1) @ Wc2 + bc2).
    hq = qf @ Wq on PE; the (K, N_q/8, 256) relu tensor is produced per
    (class, h-half) as (128h, 2048n) bf16 tiles by DVE dual-op tensor_scalar
    (relu(x+bias), 4x mode) and ACT activation (3:1 split); the Wc2-weighted
    h-reduction runs on PE as accumulating matmuls with a diag-expanded Wc2
    stationary, col-tiled 4 ways (tile_position) so four classes' matmuls
    overlap in the array.  sigmoid-mean uses sigmoid(z) = 0.5 + 0.5*tanh(z/2):
    one ACT Tanh op per psum chunk with accum_out produces the partial sums,
    AllReduced (512B) after AllReduce #1 completes, finished with an affine
    on gpsimd.
  Host side does layout staging only: sharding, bf16 casts, transposes,
  diag-expansion of Wc2, and the inverse class permutation of the conf output.

Measured on the axon-tunneled chip: ~126-136 us NEFF exec (best 120.5), of
which ~30 us is core-start skew absorbed by AllReduce #1, ~25 us per-collective
ncfw overhead x2, and ~10 us kernel drain: local compute finishes by ~72 us;
the tail is the replicated MLP chain (on a HAM-throttled cold PE) overlapped
with AllReduce #2.  Step 0's prototype-half of the first MLP layer is
pre-accumulated during the AllReduce wait; per-step wmean transposes are
emitted inside the step loop so they fill step-boundary PE gaps; the AR1
staging DMA issues from the (idle) scalar engine, which triggers the
collective ~10 us earlier than issuing it from gpsimd.
"""

import sys

for _p in ("/opt/trn_rl_repo",):
    if _p not in sys.path:
        sys.path.append(_p)

import numpy as np
import ml_dtypes

import concourse.bass as bass
import concourse.bacc as bacc
import concourse.mybir as mybir
import concourse.tile as tile
from concourse.tile_rust import add_dep_helper
from concourse.bass_utils import run_bass_kernel_spmd

F32 = mybir.dt.float32
BF16 = mybir.dt.bfloat16
BF = ml_dtypes.bfloat16

N_CORES = 8
K = 32          # classes
NQ = 16384      # queries total
C = 512         # feature dim
HR = 512        # refinement hidden
HC = 256        # confidence hidden
S = 3           # refinement steps
NQS = NQ // N_CORES   # 2048 queries per core
NB = NQS // 128       # 16 blocks of 128 queries
NCH = NQS // 512      # 4 chunks of 512 queries (psum-bank sized)

TRACE = False           # set by test harness for profiling runs
TRACE_KWARGS = {}

_CACHE = {}

AF = mybir.ActivationFunctionType
OP = mybir.AluOpType


def _build(gate_t2=True, stage_scalar=True):
    nc = bacc.Bacc("TRN2", target_bir_lowering=False, debug=False,
                   num_devices=N_CORES)

    # ---------------- dram parameters (per-core shards / replicated) ---------
    P = lambda name, shape, dt: nc.declare_dram_parameter(name, list(shape), dt, isOutput=False)
    qfn_d = P("qfn", (128, NB, C), BF16)          # qf shard, n = p*16+g
    qft_d = P("qft", (128, C // 128, NQS), BF16)  # qf shard transposed [c%128, c//128, (g,p)]
    dist_d = P("dist", (128, NB, K), F32)         # distances shard
    w1_d = P("w1", (128, 8, HR), BF16)            # W1 [cc%128, cc//128, h]
    w2_d = P("w2", (128, 4, C), BF16)             # W2 [h%128, h//128, c]
    wq_d = P("wq", (128, 4, HC), BF16)            # Wc1[512:] blocks
    wp_d = P("wp", (128, 4, HC), BF16)            # Wc1[:512] blocks
    w2d_d = P("w2d", (128, 2, K, K), BF16)        # diag-expanded Wc2
    ptT_d = P("ptT", (128, 4, K), BF16)           # prototypes^T blocks
    pt_d = P("pt", (K, C), F32)                   # prototypes (refined init)
    b1_d = P("b1r", (1, HR), BF16)
    b2_d = P("b2r", (1, C), BF16)
    bc1_d = P("bc1r", (1, HC), BF16)
    bc2_d = P("bc2c", (128, 1), F32)              # bc2 replicated down partitions
    id_d = P("ident", (128, 128), BF16)
    idf_d = P("identf", (128, 128), F32)

    ref_out = nc.declare_dram_parameter("refined", [K, C], F32, isOutput=True)
    conf_out = nc.declare_dram_parameter("conf", [128, 1], F32, isOutput=True)

    with tile.TileContext(nc) as tc:
        with (
            tc.tile_pool(name="sbuf", bufs=1) as sb,
            tc.tile_pool(name="rpool", bufs=10) as rp,
            tc.tile_pool(name="psA", bufs=4, space="PSUM") as psA,
            tc.tile_pool(name="psH", bufs=2, space="PSUM") as psH,
            tc.tile_pool(name="psG", bufs=2, space="PSUM") as psG,
            tc.tile_pool(name="dram", bufs=1, space="DRAM") as dram,
        ):
            # PSUM budget (8 banks): conf_ps 4 + hq_ps 2 + ps_gen 2
            gen_tile = lambda shape, dt: psG.tile(list(shape), dt, tag="ps_gen",
                                                  name="ps_gen")
            # ------------- input DMA to SBUF ---------------------------------
            def load(dparam, shape, dt, name):
                t = sb.tile(list(shape), dt, tag=name)
                nc.sync.dma_start(out=t[:], in_=dparam[:])
                return t

            dist = load(dist_d, (128, NB, K), F32, "dist")
            qfn = sb.tile([128, NB, C], BF16, tag="qfn", name="qfn")
            for _c in range(4):
                nc.sync.dma_start(out=qfn[:, 4 * _c:4 * (_c + 1), :],
                                  in_=qfn_d[:, 4 * _c:4 * (_c + 1), :])
            qft = sb.tile([128, 4, NQS], BF16, tag="qft", name="qft")
            for _c in range(4):
                nc.sync.dma_start(out=qft[:, :, 512 * _c:512 * (_c + 1)],
                                  in_=qft_d[:, :, 512 * _c:512 * (_c + 1)])
            wq = load(wq_d, (128, 4, HC), BF16, "wq")
            wp = load(wp_d, (128, 4, HC), BF16, "wp")
            ptT = load(ptT_d, (128, 4, K), BF16, "ptT")
            bc1r = load(bc1_d, (1, HC), BF16, "bc1r")
            bc2c = load(bc2_d, (128, 1), F32, "bc2c")
            w2d = load(w2d_d, (128, 2, K, K), BF16, "w2d")
            ident = load(id_d, (128, 128), BF16, "ident")
            identf = load(idf_d, (128, 128), F32, "identf")
            w1 = load(w1_d, (128, 8, HR), BF16, "w1")
            w2 = load(w2_d, (128, 4, C), BF16, "w2")
            b1r = load(b1_d, (1, HR), BF16, "b1r")
            b2r = load(b2_d, (1, C), BF16, "b2r")

            ones_row = sb.tile([1, K], BF16, tag="ones_row")
            nc.vector.memset(ones_row[:], 1.0)
            ones_col = sb.tile([128, 1], BF16, tag="ones_col")
            nc.vector.memset(ones_col[:], 1.0)

            # ------------- stage A: softmax + weighted-sum partials ----------
            esb = sb.tile([128, NB, S, K], F32, tag="esb")
            for s in range(S):
                nc.scalar.activation(esb[:, :, s, :], dist[:], AF.Exp,
                                     scale=-1.0 / (s + 1.0))
            zsb = sb.tile([128, NB, S], F32, tag="zsb")
            rz = sb.tile([128, NB, S], F32, tag="rz")
            soft = sb.tile([128, NB, S, K], BF16, tag="soft")
            for s in range(S):
                nc.vector.tensor_reduce(zsb[:, :, s], esb[:, :, s, :],
                                        axis=mybir.AxisListType.X, op=OP.add)
                nc.vector.reciprocal(rz[:, :, s], zsb[:, :, s])
                rz_ap = rz[:, :, s:s + 1]
                rz_b = bass.AP(rz_ap.tensor, rz_ap.offset,
                               [rz_ap.ap[0], rz_ap.ap[1], [0, K]])
                nc.vector.tensor_tensor(out=soft[:, :, s, :],
                                        in0=esb[:, :, s, :], in1=rz_b,
                                        op=OP.mult)

            num_ps = gen_tile([S * K, C], F32)
            wsum_ps = gen_tile([S * K, 1], F32)
            for g in range(NB):
                nc.tensor.matmul(wsum_ps[:], soft[:, g], ones_col[:],
                                 start=(g == 0), stop=(g == NB - 1))
            for g in range(NB):
                nc.tensor.matmul(num_ps[:], soft[:, g], qfn[:, g, :],
                                 start=(g == 0), stop=(g == NB - 1))

            # pack [num | wsum] into one bf16 staging tile -> one DMA, and
            # AllReduce on half the bytes (bf16 CCE adds are plenty accurate
            # for a 2e-2 tolerance).
            nw_sb = sb.tile([S * K, C + 1], BF16, tag="nw_sb")
            nc.vector.tensor_copy(nw_sb[:, C:], wsum_ps[:])
            nc.vector.tensor_copy(nw_sb[:, :C], num_ps[:])

            # ------------- AllReduce #1 (98KB) -------------------------------
            ar1_in = dram.tile([S * K, C + 1], BF16)
            ar1_out = dram.tile([S * K, C + 1], BF16)
            (nc.scalar if stage_scalar else nc.gpsimd).dma_start(
                out=ar1_in[:], in_=nw_sb[:])
            nc.gpsimd.collective_compute(
                "AllReduce", OP.add,
                replica_groups=[list(range(N_CORES))],
                ins=[ar1_in.opt()], outs=[ar1_out.opt()],
            )
            arn = sb.tile([S * K, C + 1], BF16, tag="arn")
            _arn_i = nc.sync.dma_start(out=arn[:], in_=ar1_out[:])

            # ------------- confidence setup: hp, hq --------------------------
            # hpT blocks (128h, K) = Wp_blk.T @ protoT_blk + bc1
            hpbT = sb.tile([128, 2, K], F32, tag="hpbT")
            for hb in range(2):
                hp_ps = gen_tile([128, K], F32)
                for cb in range(4):
                    nc.tensor.matmul(hp_ps[:],
                                     wp[:, cb, hb * 128:(hb + 1) * 128],
                                     ptT[:, cb, :],
                                     start=(cb == 0), stop=False)
                nc.tensor.matmul(hp_ps[:], bc1r[:, hb * 128:(hb + 1) * 128],
                                 ones_row[:], start=False, stop=True)
                nc.vector.tensor_copy(hpbT[:, hb, :], hp_ps[:])

            # hqT (h-part, n-free) bf16
            hqt = sb.tile([128, 2, NQS], BF16, tag="hqt")
            for hb in range(2):
                for j in range(NCH):
                    hq_ps = psH.tile([128, 512], F32, tag="hq_ps")
                    for cb in range(4):
                        nc.tensor.matmul(hq_ps[:],
                                         wq[:, cb, hb * 128:(hb + 1) * 128],
                                         qft[:, cb, j * 512:(j + 1) * 512],
                                         start=(cb == 0), stop=(cb == 3))
                    nc.vector.tensor_copy(hqt[:, hb, j * 512:(j + 1) * 512],
                                          hq_ps[:])

            # ------------- confidence main loop ------------------------------
            conf_ps = [psA.tile([128, 512], F32, tag="conf_ps", name=f"conf_ps{_j}")
                       for _j in range(NCH)]
            # col-tiled: class k -> PE col-group g=k%4, psum rows 32g..32g+31
            # (its row within the group is k//4, set by the diag layout of w2d).
            # Emission interleaves the 4 col-groups so their matmuls overlap
            # in the array.
            last_dve_relu = last_act_relu = last_conf_mm = None
            cnt = {}
            for hb in range(2):
                for kq in range(0, K, 4):
                    qi = hb * 8 + kq // 4    # quad index 0..15
                    rts = []
                    for dk in range(4):
                        k = kq + dk
                        rt = rp.tile([128, NQS], BF16, tag="rt", name="rt")
                        if dk != qi % 4:
                            last_dve_relu = nc.vector.tensor_scalar(
                                out=rt[:], in0=hqt[:, hb, :],
                                scalar1=hpbT[:, hb, k:k + 1], scalar2=0.0,
                                op0=OP.add, op1=OP.max)
                        else:
                            last_act_relu = nc.scalar.activation(
                                rt[:], hqt[:, hb, :], AF.Relu,
                                bias=hpbT[:, hb, k:k + 1])
                        rts.append(rt)
                    for j in range(NCH):
                        for dk in range(4):
                            k = kq + dk
                            g = k % 4
                            c = cnt.get((j, g), 0)
                            last_conf_mm = nc.tensor.matmul(
                                conf_ps[j][32 * g:32 * (g + 1), :],
                                w2d[:, hb, k, :],
                                rts[dk][:, j * 512:(j + 1) * 512],
                                start=(c == 0),
                                stop=(c == 2 * (K // 4) - 1),
                                tile_position=(0, 32 * g),
                                skip_group_check=True)
                            cnt[(j, g)] = c + 1

            # tanh( 0.5*raw + 0.5*bc2 ), accumulate over n
            half_bc2 = sb.tile([128, 1], F32, tag="half_bc2")
            nc.vector.tensor_scalar(out=half_bc2[:], in0=bc2c[:], scalar1=0.5,
                                    scalar2=None, op0=OP.mult)
            th_scr = sb.tile([128, 512], BF16, tag="th_scr")
            tsum = sb.tile([128, NCH], F32, tag="tsum")
            for j in range(NCH):
                nc.scalar.activation(th_scr[:], conf_ps[j][:], AF.Tanh,
                                     bias=half_bc2[:], scale=0.5,
                                     accum_out=tsum[:, j:j + 1])
            tsm = sb.tile([128, 1], F32, tag="tsm")
            _tsm_i = nc.vector.tensor_reduce(tsm[:], tsum[:],
                                             axis=mybir.AxisListType.X,
                                             op=OP.add)

            # ------------- AllReduce #2: issue from DVE so it is not blocked
            # behind AR1 on the gpsimd FIFO -------------------------------
            ar2_in = dram.tile([128, 1], F32)
            ar2_out = dram.tile([128, 1], F32)
            nc.gpsimd.dma_start(out=ar2_in[:], in_=tsm[:])
            _t2_i = nc.gpsimd.collective_compute(
                "AllReduce", OP.add,
                replica_groups=[list(range(N_CORES))],
                ins=[ar2_in.opt()], outs=[ar2_out.opt()],
            )
            # fire the 2nd collective only once the 1st has fully completed
            # (an early doorbell mid-collective was once observed to stall).
            if gate_t2:
                add_dep_helper(_t2_i.ins, _arn_i.ins, sync=True,
                               reason="trigger AR2 after AR1 completion")
            ar2_sb = sb.tile([128, 1], F32, tag="ar2_sb")
            nc.sync.dma_start(out=ar2_sb[:], in_=ar2_out[:])
            conf_f = sb.tile([128, 1], F32, tag="conf_f")
            nc.gpsimd.tensor_scalar(out=conf_f[:], in0=ar2_sb[:],
                                    scalar1=0.5 / NQ, scalar2=0.5,
                                    op0=OP.mult, op1=OP.add)
            _confdma_i = nc.sync.dma_start(out=conf_out[:], in_=conf_f[:])

            # ------------- refinement MLP chain (after AR1) ------------------
            wsc = sb.tile([S * K, 1], F32, tag="wsc")
            _wsc_i = nc.vector.tensor_scalar(out=wsc[:], in0=arn[:, C:],
                                             scalar1=1e-6,
                                             scalar2=None, op0=OP.max)
            add_dep_helper(_wsc_i.ins, last_dve_relu.ins, sync=False,
                           reason="keep AR1-gated DVE chain after relu stream")
            add_dep_helper(_wsc_i.ins, _tsm_i.ins, sync=False,
                           reason="AR2-feeding reduce before AR1-gated DVE chain")
            rws = sb.tile([S * K, 1], F32, tag="rws")
            nc.vector.reciprocal(rws[:], wsc[:])
            wmean = sb.tile([S * K, C], BF16, tag="wmean")
            nc.vector.tensor_scalar(out=wmean[:], in0=arn[:, :C],
                                    scalar1=rws[:], scalar2=None, op0=OP.mult)
            wmT = sb.tile([128, S, 4, K], BF16, tag="wmT", name="wmT")

            def emit_wmT(s):
                for cb in range(4):
                    tr_ps = gen_tile([128, K], BF16)
                    _t_i = nc.tensor.transpose(
                        tr_ps[:],
                        wmean[s * K:(s + 1) * K, cb * 128:(cb + 1) * 128],
                        ident[s * K:(s + 1) * K, s * K:(s + 1) * K])
                    if s == 0 and cb == 0:
                        add_dep_helper(_t_i.ins, last_conf_mm.ins, sync=False,
                                       reason="MLP PE work after conf stream")
                    nc.vector.tensor_copy(wmT[:, s, cb, :], tr_ps[:])

            # Step-0 refined half of the first MLP layer: refined_0 ==
            # prototypes, so these matmuls have no AR1 dependency — run them
            # during the AR1 wait; the wmean half accumulates on top later.
            h_ps0 = psH.tile([K, HR], F32, tag="hq_ps", name="h_ps0")
            for cb in range(4):
                nc.tensor.matmul(h_ps0[:], ptT[:, cb, :], w1[:, cb, :],
                                 start=(cb == 0), stop=False)
            nc.tensor.matmul(h_ps0[:], ones_row[:], b1r[:],
                             start=False, stop=False)

            ref_f = []
            for s in range(S + 1):
                ref_f.append(sb.tile([K, C], F32, tag=f"ref_f{s}", name=f"ref_f{s}"))
            nc.sync.dma_start(out=ref_f[0][:], in_=pt_d[:])

            for s in range(S):
                emit_wmT(s)
                if s == 0:
                    # refined half + b1 already accumulated in h_ps0
                    h_ps = h_ps0
                else:
                    # concat^T refined blocks: transpose the f32 state
                    # directly (fp32 PE transpose), cast in the psum->sbuf
                    # copy — no separate bf16 cast of refined needed.
                    catT = sb.tile([128, 4, K], BF16, tag="catT", name="catT")
                    for cb in range(4):
                        tr_ps = gen_tile([128, K], F32)
                        nc.tensor.transpose(
                            tr_ps[:],
                            ref_f[s][:, cb * 128:(cb + 1) * 128],
                            identf[:K, :K])
                        nc.vector.tensor_copy(catT[:, cb, :], tr_ps[:])
                    h_ps = gen_tile([K, HR], F32)
                    for cb in range(4):
                        nc.tensor.matmul(h_ps[:], catT[:, cb, :],
                                         w1[:, cb, :],
                                         start=(cb == 0), stop=False)
                    nc.tensor.matmul(h_ps[:], ones_row[:], b1r[:],
                                     start=False, stop=False)
                for cb in range(4):
                    nc.tensor.matmul(h_ps[:], wmT[:, s, cb, :],
                                     w1[:, 4 + cb, :],
                                     start=False, stop=(cb == 3))
                h_bf = sb.tile([K, HR], BF16, tag="h_bf")
                _hr_i = nc.scalar.activation(h_bf[:], h_ps[:], AF.Relu)
                if s == 0:
                    add_dep_helper(_hr_i.ins, last_act_relu.ins, sync=False,
                                   reason="MLP ACT work after relu stream")
                # hT blocks
                hT = sb.tile([128, 4, K], BF16, tag="hT")
                for cb in range(4):
                    tr_ps = gen_tile([128, K], BF16)
                    nc.tensor.transpose(tr_ps[:],
                                        h_bf[:, cb * 128:(cb + 1) * 128],
                                        ident[:K, :K])
                    nc.vector.tensor_copy(hT[:, cb, :], tr_ps[:])
                # refinement = h @ W2 + b2 ; refined += 0.1*refinement
                rf_ps = gen_tile([K, C], F32)
                for cb in range(4):
                    nc.tensor.matmul(rf_ps[:], hT[:, cb, :], w2[:, cb, :],
                                     start=(cb == 0), stop=False)
                nc.tensor.matmul(rf_ps[:], ones_row[:], b2r[:],
                                 start=False, stop=True)
                nc.vector.scalar_tensor_tensor(out=ref_f[s + 1][:],
                                               in0=rf_ps[:], scalar=0.1,
                                               in1=ref_f[s][:],
                                               op0=OP.mult, op1=OP.add)
            _refdma_i = nc.sync.dma_start(out=ref_out[:], in_=ref_f[S][:])
            add_dep_helper(_refdma_i.ins, _confdma_i.ins, sync=False,
                           reason="conf output DMA ahead of refined in sync FIFO")

    nc.compile()
    return nc


def _prep_inputs(prototypes, query_features, query_distances,
                 W1, b1, W2, b2, Wc1, bc1, Wc2, bc2):
    """Host-side sharding + layout staging (no cross-tensor arithmetic)."""
    f32 = np.float32
    qf = np.asarray(query_features, f32)
    qd = np.asarray(query_distances, f32)
    W1 = np.asarray(W1, f32); b1 = np.asarray(b1, f32)
    W2 = np.asarray(W2, f32); b2 = np.asarray(b2, f32)
    Wc1 = np.asarray(Wc1, f32); bc1 = np.asarray(bc1, f32)
    Wc2 = np.asarray(Wc2, f32); bc2 = np.asarray(bc2, f32)
    pt = np.asarray(prototypes, f32)

    def blk(a, nb):  # (nb*128, m) -> (128, nb, m)
        n, m = a.shape
        return np.ascontiguousarray(
            a.reshape(nb, 128, m).transpose(1, 0, 2)).astype(BF)

    shared = {
        "w1": blk(W1, 8),
        "w2": blk(W2, 4),
        "wq": blk(Wc1[C:], 4),
        "wp": blk(Wc1[:C], 4),
        "ptT": blk(np.ascontiguousarray(pt.T), 4),
        "pt": pt,
        "b1r": b1.reshape(1, HR).astype(BF),
        "b2r": b2.reshape(1, C).astype(BF),
        "bc1r": bc1.reshape(1, HC).astype(BF),
        "bc2c": np.full((128, 1), float(bc2.reshape(-1)[0]), f32),
        "ident": np.eye(128, dtype=f32).astype(BF),
        "identf": np.eye(128, dtype=f32),
    }
    w2dm = np.zeros((128, 2, K, K), f32)
    for hb in range(2):
        for kk in range(K):
            w2dm[:, hb, kk, kk // 4] = Wc2[hb * 128:(hb + 1) * 128, 0]
    shared["w2d"] = w2dm.astype(BF)

    in_maps = []
    for s in range(N_CORES):
        qf_sh = qf[s * NQS:(s + 1) * NQS]          # (2048, 512)
        qd_sh = qd[s * NQS:(s + 1) * NQS]          # (2048, 32)
        qfn = qf_sh.reshape(128, NB, C).astype(BF)  # n = p*16+g
        # qft[c%128, c//128, g*128+p] = qf_sh[p*16+g, c]
        qft = np.ascontiguousarray(
            qf_sh.reshape(128, NB, C).transpose(2, 1, 0)  # (C, NB, 128)
            .reshape(C, NQS).reshape(4, 128, NQS).transpose(1, 0, 2)
        ).astype(BF)
        dist = np.ascontiguousarray(qd_sh.reshape(128, NB, K))
        m = dict(shared)
        m.update({"qfn": qfn, "qft": qft, "dist": dist})
        in_maps.append(m)
    return in_maps


def kernel(**inputs):
    if "nc" not in _CACHE:
        _CACHE["nc"] = _build()
    nc = _CACHE["nc"]
    in_maps = _prep_inputs(**inputs)
    res = run_bass_kernel_spmd(nc, in_maps, core_ids=list(range(N_CORES)),
                               trace=TRACE, **TRACE_KWARGS)
    _CACHE["last_result"] = res
    refined = np.asarray(res.results[0]["refined"], np.float32).reshape(K, C)
    conf_raw = np.asarray(res.results[0]["conf"], np.float32).reshape(128)
    kk = np.arange(K)
    conf = conf_raw[32 * (kk % 4) + kk // 4]
    return refined, conf
